# revision 1
# baseline (speedup 1.0000x reference)
"""Trainium2 Bass kernel for nn_BasicNCAModel (neural cellular automaton).

Strategy (pure data parallelism, batch 16 -> 2 images per core x 8 cores):

* State layout per core: [128 partitions = 2 images x 64 channels,
  130 x 130 reflect-padded spatial grid] in SBUF, float32r, ping-pong
  double-buffered across the 8 NCA steps.
* The two depthwise 3x3 convs never materialize: they are folded into the
  hidden-layer matmul.  h = relu(W x + A1 conv1(x) + A2 conv2(x) + b)
  = relu(sum_tap E_tap @ x_shifted(tap) + b) with host-precomputed
  E_tap[256, 64]; the 9 taps accumulate in PSUM (K=64 each, the two
  images run concurrently as PE row-tiles 0-63 / 64-127).
* Stochastic fire gate and the life mask are precomputed on the host
  ((rand > 0.5) and the life mask is static: channel 0 is immutable, so
  life == (x0_init > 0) for every step) and applied per pixel via a K=1
  bf16 broadcast matmul + DVE multiply.
* float32r (13-bit mantissa, full-precision fp32 PSUM accumulation) gives
  1 PE cycle/column instead of fp32's 4.
"""
import sys
sys.path.insert(0, '/opt/trn_rl_repo')

import numpy as np

B, H, W, C = 16, 128, 128, 64
HID = 256
STEPS = 8
NCORES = 8
BPC = B // NCORES            # images per core = 2
WP, HP = W + 2, H + 2        # padded grid 130 x 130
RPG = 4                      # W-rows per group
NPIX = RPG * H               # 512 pixels per matmul tile
NG = W // RPG                # 32 groups per step

_nc_cache = {}


def _round_f32r(a):
    """Round-to-nearest-even fp32 -> fp32r (12 explicit mantissa bits)."""
    u = np.ascontiguousarray(a, np.float32).view(np.uint32)
    r = (u + 0x7FF + ((u >> 12) & 1)) & np.uint32(0xFFFFF000)
    return r.view(np.float32)


def _build():
    import concourse.bacc as bacc
    import concourse.mybir as mybir
    import concourse.tile as tile

    F32 = mybir.dt.float32
    F32R = mybir.dt.float32r
    BF16 = mybir.dt.bfloat16
    AF = mybir.ActivationFunctionType
    ALU = mybir.AluOpType

    nc = bacc.Bacc("TRN2", target_bir_lowering=False, debug=False,
                   enable_asserts=False, num_devices=NCORES)

    X0 = nc.dram_tensor("x0", [128, WP, HP], F32R, kind="ExternalInput")
    WT = nc.dram_tensor("wt", [128, 9, 2, 128], F32R, kind="ExternalInput")
    WF = nc.dram_tensor("wf", [128, 2, 64], BF16, kind="ExternalInput")
    BI = nc.dram_tensor("bi", [128, 2], F32, kind="ExternalInput")
    GL = nc.dram_tensor("gl", [STEPS, NG, 2, NPIX], BF16, kind="ExternalInput")
    LG = nc.dram_tensor("lg", [NG, 2, NPIX], BF16, kind="ExternalInput")
    OUT = nc.dram_tensor("out", [128, W, H], F32, kind="ExternalOutput")

    with tile.TileContext(nc) as tc:
        with tc.tile_pool(name="const", bufs=1) as cp, \
             tc.tile_pool(name="hbuf", bufs=2) as hp, \
             tc.tile_pool(name="gbuf", bufs=3) as gp, \
             tc.tile_pool(name="ph", bufs=1, space="PSUM") as php, \
             tc.tile_pool(name="pdx", bufs=2, space="PSUM") as pdxp:

            xA = cp.tile([128, WP, HP], F32R, tag="xA")
            xB = cp.tile([128, WP, HP], F32R, tag="xB")
            wt = cp.tile([128, 9, 2, 128], F32R, tag="wt")
            wf = cp.tile([128, 2, 64], BF16, tag="wf")
            bi = cp.tile([128, 2], F32, tag="bi")

            for c in range(4):
                r0, r1 = (WP * c) // 4, (WP * (c + 1)) // 4
                nc.sync.dma_start(xA[:, r0:r1, :], X0[:, r0:r1, :])
            nc.sync.dma_start(wt[:], WT[:])
            nc.sync.dma_start(wf[:], WF[:])
            nc.sync.dma_start(bi[:], BI[:])

            TAPS = [(a, b) for a in range(3) for b in range(3)]

            def emit_tail(p):
                """mm2 + gate-multiply + state update for a completed group
                (issued one group later so relu/gate never stall the PE)."""
                hA, hB, gateA, gateB, lbs, xs, xd, w0, t0, last = p
                dx = pdxp.tile([128, NPIX], F32, tag="dx")
                for k in range(2):
                    nc.tensor.matmul(dx[0:64, :], wf[:, k, :], hA[:, k, :],
                                     start=k == 0, stop=k == 1,
                                     skip_group_check=True)
                    nc.tensor.matmul(dx[64:128, :], wf[:, k, :], hB[:, k, :],
                                     start=k == 0, stop=k == 1,
                                     skip_group_check=True,
                                     tile_position=(0, 64))
                tg = hp.tile([128, NPIX], F32, tag="tg")
                nc.vector.tensor_tensor(tg[0:64, :], dx[0:64, :],
                                        gateA[0:64, :], ALU.mult)
                nc.vector.tensor_tensor(tg[64:128, :], dx[64:128, :],
                                        gateB[64:128, :], ALU.mult)
                tg3 = tg[:].rearrange("p (a b) -> p a b", a=RPG)
                src_i = xs[:, w0 + 1:w0 + 1 + RPG, 1:1 + H]
                dst_i = xd[:, w0 + 1:w0 + 1 + RPG, 1:1 + H]
                if t0:
                    # x1 = x0*L + dx*GL   (GL already includes L)
                    lbA, lbB = lbs
                    t2 = hp.tile([128, NPIX], F32, tag="tg2")
                    t23 = t2[:].rearrange("p (a b) -> p a b", a=RPG)
                    nc.vector.tensor_tensor(t2[0:64, :].rearrange("p (a b) -> p a b", a=RPG),
                                            xs[0:64, w0 + 1:w0 + 1 + RPG, 1:1 + H],
                                            lbA[0:64, :].rearrange("p (a b) -> p a b", a=RPG),
                                            ALU.mult)
                    nc.vector.tensor_tensor(t2[64:128, :].rearrange("p (a b) -> p a b", a=RPG),
                                            xs[64:128, w0 + 1:w0 + 1 + RPG, 1:1 + H],
                                            lbB[64:128, :].rearrange("p (a b) -> p a b", a=RPG),
                                            ALU.mult)
                    nc.vector.tensor_tensor(dst_i, tg3, t23, ALU.add)
                else:
                    nc.vector.tensor_tensor(dst_i, tg3, src_i, ALU.add)
                if last:
                    nc.sync.dma_start(OUT[:, w0:w0 + RPG, :], dst_i.bitcast(F32))

            pend = None
            for t in range(STEPS):
                xs, xd = (xA, xB) if t % 2 == 0 else (xB, xA)
                for g in range(NG):
                    w0 = RPG * g

                    # fire(+life) gate for this group's pixels: DMA rows, then
                    # broadcast across channel partitions on the idle GpSimd
                    glA = gp.tile([1, NPIX], BF16, tag="glA")
                    glB = gp.tile([1, NPIX], BF16, tag="glB")
                    nc.sync.dma_start(glA[:], GL[t, g, 0:1, :])
                    nc.sync.dma_start(glB[:], GL[t, g, 1:2, :])
                    gateA = hp.tile([128, NPIX], BF16, tag="gateA")
                    gateB = hp.tile([128, NPIX], BF16, tag="gateB")
                    nc.gpsimd.partition_broadcast(gateA[:], glA[:])
                    nc.gpsimd.partition_broadcast(gateB[:], glB[:])

                    lbs = None
                    if t == 0:
                        lgA = gp.tile([1, NPIX], BF16, tag="lgA")
                        lgB = gp.tile([1, NPIX], BF16, tag="lgB")
                        nc.sync.dma_start(lgA[:], LG[g, 0:1, :])
                        nc.sync.dma_start(lgB[:], LG[g, 1:2, :])
                        lbA = hp.tile([128, NPIX], BF16, tag="lbA")
                        lbB = hp.tile([128, NPIX], BF16, tag="lbB")
                        nc.gpsimd.partition_broadcast(lbA[:], lgA[:])
                        nc.gpsimd.partition_broadcast(lbB[:], lgB[:])
                        lbs = (lbA, lbB)

                    # --- mm1: folded conv + hidden layer, 9 taps x 2 M-chunks,
                    #     images A/B as concurrent PE row-tiles; m-chunks kept
                    #     blocked so relu(m0) overlaps the m1 taps, and the
                    #     previous group's mm2 slots in between the two blocks
                    phs = [[php.tile([128, NPIX], F32, tag=f"ph{im}{m}",
                                     name=f"ph{im}{m}")
                            for m in range(2)] for im in range(2)]
                    hA = hp.tile([128, 2, NPIX], BF16, tag="hA")
                    hB = hp.tile([128, 2, NPIX], BF16, tag="hB")
                    for m in range(2):
                        for ti, (a, b) in enumerate(TAPS):
                            rhsA = xs[0:64, w0 + a:w0 + a + RPG, b:b + H]
                            rhsB = xs[64:128, w0 + a:w0 + a + RPG, b:b + H]
                            st = ti == 0
                            sp = ti == len(TAPS) - 1
                            nc.tensor.matmul(phs[0][m][:], wt[0:64, ti, m, :], rhsA,
                                             start=st, stop=sp, skip_group_check=True)
                            nc.tensor.matmul(phs[1][m][:], wt[64:128, ti, m, :], rhsB,
                                             start=st, stop=sp, skip_group_check=True)
                        # relu + bias for this chunk, PSUM -> SBUF (bf16 for mm2)
                        nc.scalar.activation(hA[:, m, :], phs[0][m][:], AF.Relu,
                                             bias=bi[:, m:m + 1])
                        nc.scalar.activation(hB[:, m, :], phs[1][m][:], AF.Relu,
                                             bias=bi[:, m:m + 1])
                        if m == 0 and pend is not None:
                            emit_tail(pend)
                            pend = None

                    if pend is not None:
                        emit_tail(pend)
                    pend = (hA, hB, gateA, gateB, lbs, xs, xd, w0, t == 0, t == STEPS - 1)

                # finish the last group, then refresh the reflect halo of xd
                # (cols first, then rows -> corners ok)
                emit_tail(pend)
                pend = None
                if t == STEPS - 1:
                    continue
                nc.vector.tensor_copy(xd[:, :, 0], xd[:, :, 2])
                nc.vector.tensor_copy(xd[:, :, HP - 1], xd[:, :, HP - 3])
                nc.vector.tensor_copy(xd[:, 0, :], xd[:, 2, :])
                nc.vector.tensor_copy(xd[:, WP - 1, :], xd[:, WP - 3, :])


    nc.compile()
    return nc


def _host_pack(x, w_conv1, w_conv2, w_hidden, b_hidden, w_final, rand_vals):
    import ml_dtypes
    bf16 = ml_dtypes.bfloat16

    Wh = np.asarray(w_hidden, np.float64)            # [256, 192]
    w1 = np.asarray(w_conv1, np.float64)[:, 0]       # [64, 3, 3]
    w2 = np.asarray(w_conv2, np.float64)[:, 0]

    wtaps = np.zeros((128, 9, 2, 128), np.float32)
    for ti, (a, b) in enumerate([(a, b) for a in range(3) for b in range(3)]):
        E = Wh[:, 64:128] * w1[None, :, a, b] + Wh[:, 128:192] * w2[None, :, a, b]
        if (a, b) == (1, 1):
            E = E + Wh[:, 0:64]
        for m in range(2):
            lhsT = E[128 * m:128 * (m + 1), :].T.astype(np.float32)   # [64, 128]
            wtaps[0:64, ti, m, :] = lhsT
            wtaps[64:128, ti, m, :] = lhsT
    wtaps = _round_f32r(wtaps)

    wfz = np.asarray(w_final, np.float32).copy()     # [64, 256]
    wfz[0:4, :] = 0.0                                # immutable image channels
    wfT = wfz.T                                      # [256, 64]
    wf = np.stack([wfT[0:128], wfT[128:256]], axis=1)          # [128, 2, 64]
    wf = np.ascontiguousarray(wf).astype(bf16)

    bi = np.stack([b_hidden[0:128], b_hidden[128:256]], axis=1).astype(np.float32)

    # life mask is static: channel-0 updates are masked out, so life(t) == (x0_init > 0)
    Lhw = np.asarray(x)[..., 0] > 0                  # [B, H, W]
    Lwh = np.ascontiguousarray(Lhw.transpose(0, 2, 1))   # [B, W, H]
    G = np.asarray(rand_vals)[..., 0] > 0.5          # [S, B, H, W]
    GLw = G.transpose(0, 1, 3, 2) & Lwh[None]        # [S, B, W, H]

    x_chw = np.asarray(x, np.float32).transpose(0, 3, 2, 1)      # [B, C, W, H]
    xp = np.pad(x_chw, ((0, 0), (0, 0), (1, 1), (1, 1)), mode='reflect')
    xp = _round_f32r(np.ascontiguousarray(xp))

    in_maps = []
    for i in range(NCORES):
        sl = slice(BPC * i, BPC * (i + 1))
        x0 = np.ascontiguousarray(xp[sl].reshape(BPC * C, WP, HP))
        glc = np.ascontiguousarray(
            GLw[:, sl].reshape(STEPS, BPC, NG, NPIX).transpose(0, 2, 1, 3)
        ).astype(bf16)
        lgc = np.ascontiguousarray(
            Lwh[sl].reshape(BPC, NG, NPIX).transpose(1, 0, 2)
        ).astype(bf16)
        in_maps.append({
            "x0": x0, "wt": wtaps, "wf": wf, "bi": bi,
            "gl": glc, "lg": lgc,
        })
    return in_maps


def _run(inputs, trace=False, trace_kwargs=None):
    from concourse.bass_utils import run_bass_kernel_spmd
    if "nc" not in _nc_cache:
        _nc_cache["nc"] = _build()
    nc = _nc_cache["nc"]
    in_maps = _host_pack(
        inputs["x"], inputs["w_conv1"], inputs["w_conv2"], inputs["w_hidden"],
        inputs["b_hidden"], inputs["w_final"], inputs["rand_vals"])
    kwargs = {}
    if trace:
        kwargs["trace"] = True
        if trace_kwargs:
            kwargs.update(trace_kwargs)
    res = run_bass_kernel_spmd(nc, in_maps, core_ids=list(range(NCORES)), **kwargs)
    outs = []
    for i in range(NCORES):
        o = res.results[i]["out"].reshape(BPC, C, W, H)
        outs.append(o.transpose(0, 3, 2, 1))         # -> [b, H, W, C]
    full = np.concatenate(outs, axis=0).astype(np.float32)
    return full, res


def kernel(**inputs) -> np.ndarray:
    steps = int(np.asarray(inputs.get("steps", STEPS)))
    assert steps == STEPS, f"kernel compiled for {STEPS} steps, got {steps}"
    out, _ = _run(inputs)
    return out

